# revision 7
# baseline (speedup 1.0000x reference)
"""Trainium2 Bass kernel for an autoregressive decoder layer (decode step).

Shapes (full): B=1024, E=128, H=8 heads x HD=16, cross-attn ctx N1=1001,
self-attn KV cache T_PREV=511 (+1 computed token -> 512).

Sharding: pure data parallel over 8 NeuronCores; 128 batches per core,
weights replicated. No collectives. On-chip layout: partition = local batch.

Head semantics (faithful to the reference's raw reshape [B,S,E]->[B*H,S,HD]):
head h of a key/value buffer reads the flat (S*E) per-batch buffer at
offsets h*S*HD + t*HD + d. The query (S=1) uses the clean E-slice per head.
"""

import sys
from contextlib import ExitStack

import numpy as np

if "/opt/trn_rl_repo" not in sys.path:
    sys.path.insert(0, "/opt/trn_rl_repo")

import concourse.bacc as bacc
import concourse.bass as bass
import concourse.mybir as mybir
from concourse.tile import TileContext
from concourse.bass_utils import run_bass_kernel_spmd
from concourse.masks import make_identity

F32 = mybir.dt.float32
U8 = mybir.dt.uint8

B = 1024
E = 128
H = 8
HD = 16
N1 = 1001
T_PREV = 511
NCORES = 8
BL = B // NCORES  # 128 batches per core
EPS = 1e-5
TH = 128  # seq positions per (head, tile)

WNAMES = ["Wk", "Wv", "W0sa", "Wqatt", "W0att", "W1", "W2"]


def build_kernel(bl=BL, n1=N1, t_prev=T_PREV, repeat=1):
    nc = bacc.Bacc("TRN2", target_bir_lowering=False, debug=False,
                   num_devices=NCORES)

    # ---- dram parameters ----
    d_ht = nc.declare_dram_parameter("h_t", [bl, E], F32, isOutput=False)
    d_katt = nc.declare_dram_parameter("K_att", [bl, n1, E], F32, isOutput=False)
    d_vatt = nc.declare_dram_parameter("V_att", [bl, n1, E], F32, isOutput=False)
    d_ksa = nc.declare_dram_parameter("K_sa_prev", [bl, t_prev, E], F32, isOutput=False)
    d_vsa = nc.declare_dram_parameter("V_sa_prev", [bl, t_prev, E], F32, isOutput=False)
    d_mask = nc.declare_dram_parameter("mask", [bl, n1], U8, isOutput=False)
    d_w = {}
    d_b = {}
    for w in WNAMES:
        d_w[w] = nc.declare_dram_parameter(w + "_w", [E, E], F32, isOutput=False)
        d_b[w] = nc.declare_dram_parameter(w + "_b", [1, E], F32, isOutput=False)
    d_lng = {}
    d_lnb = {}
    for ln in ["ln_sa", "ln_ff"]:
        d_lng[ln] = nc.declare_dram_parameter(ln + "_g", [1, E], F32, isOutput=False)
        d_lnb[ln] = nc.declare_dram_parameter(ln + "_b", [1, E], F32, isOutput=False)
    d_out = nc.declare_dram_parameter("out", [bl, E], F32, isOutput=True)

    with TileContext(nc) as tc, ExitStack() as ctx:
        const = ctx.enter_context(tc.tile_pool(name="const", bufs=1))
        xpool = ctx.enter_context(tc.tile_pool(name="xpool", bufs=2))
        kpool = ctx.enter_context(tc.tile_pool(name="kpool", bufs=3))
        vpool = ctx.enter_context(tc.tile_pool(name="vpool", bufs=3))
        ppool = ctx.enter_context(tc.tile_pool(name="ppool", bufs=3))
        spool = ctx.enter_context(tc.tile_pool(name="spool", bufs=3))
        acc = ctx.enter_context(tc.tile_pool(name="acc", bufs=2))
        small = ctx.enter_context(tc.tile_pool(name="small", bufs=4))
        psum = ctx.enter_context(tc.tile_pool(name="psum", bufs=2, space="PSUM"))

        # ---- constants ----
        ident = const.tile([128, 128], F32)
        make_identity(nc, ident[:])
        eps_t = const.tile([128, 1], F32)
        nc.vector.memset(eps_t[:], EPS)

        # weights transposed to [e_in, e_out]; biases broadcast to [128, E]
        wt = {}
        bfull = {}
        for w in WNAMES:
            wsb = xpool.tile([E, E], F32, tag="wstage")
            nc.sync.dma_start(out=wsb[:], in_=d_w[w][:])
            pst = psum.tile([E, E], F32, tag="pst")
            nc.tensor.transpose(pst[:], wsb[:], ident[:])
            wt[w] = const.tile([E, E], F32, tag="wt_" + w, name="wt_" + w)
            nc.any.tensor_copy(wt[w][:], pst[:])
            bfull[w] = const.tile([128, E], F32, tag="bf_" + w, name="bf_" + w)
            nc.gpsimd.dma_start(out=bfull[w][:],
                                in_=d_b[w].ap().partition_broadcast(128))
        lngf = {}
        lnbf = {}
        for ln in ["ln_sa", "ln_ff"]:
            lngf[ln] = const.tile([128, E], F32, tag="lng_" + ln, name="lng_" + ln)
            nc.gpsimd.dma_start(out=lngf[ln][:],
                                in_=d_lng[ln].ap().partition_broadcast(128))
            lnbf[ln] = const.tile([128, E], F32, tag="lnb_" + ln, name="lnb_" + ln)
            nc.gpsimd.dma_start(out=lnbf[ln][:],
                                in_=d_lnb[ln].ap().partition_broadcast(128))

        # mask -> negmask f32 (-1e9 where masked)
        m8 = const.tile([128, n1], U8)
        nc.sync.dma_start(out=m8[:], in_=d_mask[:])
        negmask = const.tile([128, n1], F32)
        nc.vector.tensor_scalar(negmask[:], m8[:], -1.0e9, None,
                                mybir.AluOpType.mult)

        # h_t
        ht = const.tile([128, E], F32)
        nc.sync.dma_start(out=ht[:], in_=d_ht[:])

        # ---- helpers ----
        def linear(x, w, out, extra_add=None):
            """out = x @ W^T + b (+ extra_add). x, out: [128, E] sbuf f32."""
            pst = psum.tile([E, E], F32, tag="pst")
            nc.tensor.transpose(pst[:], x[:], ident[:])
            xt = xpool.tile([E, E], F32, tag="xt")
            nc.any.tensor_copy(xt[:], pst[:])
            yps = psum.tile([128, E], F32, tag="yps")
            nc.tensor.matmul(yps[:], xt[:], wt[w][:], start=True, stop=True)
            if extra_add is None:
                nc.vector.tensor_add(out[:], yps[:], bfull[w][:])
            else:
                tmp = xpool.tile([128, E], F32, tag="lin_tmp")
                nc.vector.tensor_add(tmp[:], yps[:], bfull[w][:])
                nc.vector.tensor_add(out[:], tmp[:], extra_add[:])

        def layernorm(x, ln, out):
            stats = small.tile([128, 6], F32, tag="bn_stats")
            nc.vector.bn_stats(stats[:], x[:])
            mv = small.tile([128, 2], F32, tag="bn_mv")
            nc.vector.bn_aggr(mv[:], stats[:])
            std = small.tile([128, 1], F32, tag="std")
            nc.scalar.activation(std[:], mv[:, 1:2],
                                 mybir.ActivationFunctionType.Sqrt,
                                 bias=eps_t[:], scale=1.0)
            rstd = small.tile([128, 1], F32, tag="rstd")
            nc.vector.reciprocal(rstd[:], std[:])
            xn = xpool.tile([128, E], F32, tag="ln_xn")
            nc.vector.tensor_scalar(xn[:], x[:], mv[:, 0:1], rstd[:],
                                    mybir.AluOpType.subtract,
                                    mybir.AluOpType.mult)
            xg = xpool.tile([128, E], F32, tag="ln_xg")
            nc.vector.tensor_mul(xg[:], xn[:], lngf[ln][:])
            nc.vector.tensor_add(out[:], xg[:], lnbf[ln][:])

        def attention(q, kd, vd, s_tot, n_prev, kv_extra, masked, a_out):
            """Batched MHA decode, raw-reshape head semantics.
            q: [128, E] sbuf. kd/vd: dram [bl, n_prev, E] flat-viewed.
            s_tot: total positions per head (n_prev, or n_prev+1 w/ extra).
            kv_extra: None or (k_new, v_new) [128, E] appended at flat end.
            a_out: [128, E] sbuf."""
            nflat_prev = n_prev * E
            ntiles = (s_tot + TH - 1) // TH
            kflat = kd[:].rearrange("b t e -> b (t e)")
            vflat = vd[:].rearrange("b t e -> b (t e)")
            dparts = acc.tile([128, H, ntiles], F32, tag="dparts")
            oparts = acc.tile([128, H, ntiles, HD], F32, tag="oparts")
            for h in range(H):
                qh = q[:, h * HD:(h + 1) * HD]
                for i in range(ntiles):
                    t0 = i * TH
                    ti = min(TH, s_tot - t0)
                    f0 = h * s_tot * HD + t0 * HD
                    avail = max(0, min(ti * HD, nflat_prev - f0))
                    nd = avail // HD  # positions from dram
                    kt = kpool.tile([128, TH, HD], F32, tag="kt")
                    vt = vpool.tile([128, TH, HD], F32, tag="vt")
                    if nd > 0:
                        nc.sync.dma_start(
                            out=kt[:, :nd, :],
                            in_=kflat[:, f0:f0 + nd * HD].rearrange(
                                "b (t d) -> b t d", d=HD))
                        nc.sync.dma_start(
                            out=vt[:, :nd, :],
                            in_=vflat[:, f0:f0 + nd * HD].rearrange(
                                "b (t d) -> b t d", d=HD))
                    if nd < ti:
                        e0 = f0 + nd * HD - nflat_prev
                        ncp = (ti - nd) * HD
                        nc.vector.tensor_copy(
                            kt[:, nd:ti, :],
                            kv_extra[0][:, e0:e0 + ncp].rearrange(
                                "b (t d) -> b t d", d=HD))
                        nc.vector.tensor_copy(
                            vt[:, nd:ti, :],
                            kv_extra[1][:, e0:e0 + ncp].rearrange(
                                "b (t d) -> b t d", d=HD))
                    # scores for head h: s[b,t] = sum_d kt[b,t,d]*qh[b,d]
                    prod = ppool.tile([128, TH, HD], F32, tag="prod")
                    qb = qh.unsqueeze(1).broadcast_to([128, ti, HD])
                    nc.vector.tensor_mul(prod[:, :ti, :], kt[:, :ti, :], qb)
                    s_raw = spool.tile([128, TH], F32, tag="s_raw")
                    nc.vector.tensor_reduce(s_raw[:, :ti], prod[:, :ti, :],
                                            mybir.AxisListType.X,
                                            mybir.AluOpType.add)
                    p = spool.tile([128, TH], F32, tag="p")
                    if masked:
                        s_msk = spool.tile([128, TH], F32, tag="s_msk")
                        nc.vector.scalar_tensor_tensor(
                            s_msk[:, :ti], s_raw[:, :ti], 0.25,
                            negmask[:, t0:t0 + ti],
                            mybir.AluOpType.mult, mybir.AluOpType.add)
                        nc.scalar.activation(p[:, :ti], s_msk[:, :ti],
                                             mybir.ActivationFunctionType.Exp)
                    else:
                        nc.scalar.activation(p[:, :ti], s_raw[:, :ti],
                                             mybir.ActivationFunctionType.Exp,
                                             scale=0.25)
                    nc.vector.tensor_reduce(dparts[:, h, i:i + 1], p[:, :ti],
                                            mybir.AxisListType.X,
                                            mybir.AluOpType.add)
                    # o[b,d] += sum_t p[b,t] * vt[b,t,d]
                    pv = ppool.tile([128, TH, HD], F32, tag="pv")
                    pb = p[:, :ti].unsqueeze(2).broadcast_to([128, ti, HD])
                    nc.vector.tensor_mul(pv[:, :ti, :], vt[:, :ti, :], pb)
                    nc.vector.tensor_reduce(oparts[:, h, i, :],
                                            pv[:, :ti, :].transpose([0, 2, 1]),
                                            mybir.AxisListType.X,
                                            mybir.AluOpType.add)
            d = small.tile([128, H], F32, tag="attn_d")
            nc.vector.tensor_reduce(d[:], dparts[:], mybir.AxisListType.X,
                                    mybir.AluOpType.add)
            r = small.tile([128, H], F32, tag="attn_r")
            nc.vector.reciprocal(r[:], d[:])
            o = xpool.tile([128, E], F32, tag="attn_o")
            nc.vector.tensor_reduce(o[:].rearrange("p (h d) -> p h d", h=H),
                                    oparts[:].transpose([0, 1, 3, 2]),
                                    mybir.AxisListType.X, mybir.AluOpType.add)
            rb = r[:].unsqueeze(2).broadcast_to([128, H, HD])
            nc.vector.tensor_mul(
                a_out[:].rearrange("p (h d) -> p h d", h=H),
                o[:].rearrange("p (h d) -> p h d", h=H), rb)

        # ---- model ----
        for _rep in range(repeat):
            k_sa = xpool.tile([128, E], F32, tag="k_sa", name="k_sa")
            linear(ht, "Wk", k_sa)
            v_sa = xpool.tile([128, E], F32, tag="v_sa", name="v_sa")
            linear(ht, "Wv", v_sa)

            a_sa = xpool.tile([128, E], F32, tag="a_sa", name="a_sa")
            attention(ht, d_ksa, d_vsa, t_prev + 1, t_prev, (k_sa, v_sa),
                      False, a_sa)

            h1 = xpool.tile([128, E], F32, tag="h1", name="h1")
            linear(a_sa, "W0sa", h1, extra_add=ht)
            h1ln = xpool.tile([128, E], F32, tag="h1ln", name="h1ln")
            layernorm(h1, "ln_sa", h1ln)

            q = xpool.tile([128, E], F32, tag="q", name="q")
            linear(h1ln, "Wqatt", q)
            a_att = xpool.tile([128, E], F32, tag="a_att", name="a_att")
            attention(q, d_katt, d_vatt, n1, n1, None, True, a_att)

            h2 = xpool.tile([128, E], F32, tag="h2", name="h2")
            linear(a_att, "W0att", h2, extra_add=h1ln)
            h2ln = xpool.tile([128, E], F32, tag="h2ln", name="h2ln")
            layernorm(h2, "ln_sa", h2ln)

            ff_pre = xpool.tile([128, E], F32, tag="ff_pre", name="ff_pre")
            linear(h2ln, "W1", ff_pre)
            ff = xpool.tile([128, E], F32, tag="ff", name="ff")
            nc.scalar.activation(ff[:], ff_pre[:],
                                 mybir.ActivationFunctionType.Relu)
            h3 = xpool.tile([128, E], F32, tag="h3", name="h3")
            linear(ff, "W2", h3, extra_add=h2ln)
            h3ln = xpool.tile([128, E], F32, tag="h3ln", name="h3ln")
            layernorm(h3, "ln_ff", h3ln)

            nc.sync.dma_start(out=d_out[:], in_=h3ln[:])

    nc.compile()
    return nc


_NC_CACHE = {}


def _get_nc():
    key = (BL, N1, T_PREV)
    if key not in _NC_CACHE:
        _NC_CACHE[key] = build_kernel()
    return _NC_CACHE[key]


def make_in_maps(inputs, bl=BL, ncores=NCORES):
    """Shard batch dim across cores; replicate weights."""
    in_maps = []
    for c in range(ncores):
        sl = slice(c * bl, (c + 1) * bl)
        m = {}
        m["h_t"] = np.ascontiguousarray(
            inputs["h_t"][sl].reshape(bl, E).astype(np.float32))
        for k in ["K_att", "V_att", "K_sa_prev", "V_sa_prev"]:
            m[k] = np.ascontiguousarray(inputs[k][sl].astype(np.float32))
        m["mask"] = np.ascontiguousarray(inputs["mask"][sl].astype(np.uint8))
        for w in WNAMES:
            m[w + "_w"] = np.ascontiguousarray(inputs[w + "_w"].astype(np.float32))
            m[w + "_b"] = np.ascontiguousarray(
                inputs[w + "_b"].reshape(1, E).astype(np.float32))
        for ln in ["ln_sa", "ln_ff"]:
            m[ln + "_g"] = np.ascontiguousarray(
                inputs[ln + "_g"].reshape(1, E).astype(np.float32))
            m[ln + "_b"] = np.ascontiguousarray(
                inputs[ln + "_b"].reshape(1, E).astype(np.float32))
        in_maps.append(m)
    return in_maps


def kernel(**inputs):
    nc = _get_nc()
    in_maps = make_in_maps(inputs)
    res = run_bass_kernel_spmd(nc, in_maps, core_ids=list(range(NCORES)))
    outs = [res.results[i]["out"].reshape(BL, 1, E) for i in range(NCORES)]
    return np.concatenate(outs, axis=0)


# revision 15
# speedup vs baseline: 2.8030x; 2.8030x over previous
"""Trainium2 Bass kernel for an autoregressive decoder layer (decode step).

Shapes (full): B=1024, E=128, H=8 heads x HD=16, cross-attn ctx N1=1001,
self-attn KV cache T_PREV=511 (+1 computed token -> 512).

Sharding: pure data parallel over 8 NeuronCores; 128 batches per core,
weights replicated. No collectives. On-chip layout: partition = local batch.

Head semantics (faithful to the reference's raw reshape [B,S,E]->[B*H,S,HD]):
head h of a key/value buffer reads the flat (S*E) per-batch buffer at
offsets h*S*HD + t*HD + d. The query (S=1) uses the clean E-slice per head.
"""

import sys
from contextlib import ExitStack

import numpy as np

if "/opt/trn_rl_repo" not in sys.path:
    sys.path.insert(0, "/opt/trn_rl_repo")

import concourse.bacc as bacc
import concourse.bass as bass
import concourse.mybir as mybir
from concourse.tile import TileContext
from concourse.bass_utils import run_bass_kernel_spmd
from concourse.masks import make_identity

F32 = mybir.dt.float32
F16 = mybir.dt.float16
U8 = mybir.dt.uint8

B = 1024
E = 128
H = 8
HD = 16
N1 = 1001
T_PREV = 511
NCORES = 8
BL = B // NCORES  # 128 batches per core
EPS = 1e-5
TH = 128  # seq positions per (head, tile)

WNAMES = ["Wk", "Wv", "W0sa", "Wqatt", "W0att", "W1", "W2"]


def build_kernel(bl=BL, n1=N1, t_prev=T_PREV, repeat=1, mode="full"):
    nc = bacc.Bacc("TRN2", target_bir_lowering=False, debug=False,
                   num_devices=NCORES)

    # ---- dram parameters ----
    d_ht = nc.declare_dram_parameter("h_t", [bl, E], F32, isOutput=False)
    d_katt = nc.declare_dram_parameter("K_att", [bl, n1, E], F32, isOutput=False)
    d_vatt = nc.declare_dram_parameter("V_att", [bl, n1, E], F32, isOutput=False)
    d_ksa = nc.declare_dram_parameter("K_sa_prev", [bl, t_prev, E], F32, isOutput=False)
    d_vsa = nc.declare_dram_parameter("V_sa_prev", [bl, t_prev, E], F32, isOutput=False)
    d_mask = nc.declare_dram_parameter("mask", [bl, n1], U8, isOutput=False)
    d_w = {}
    d_b = {}
    for w in WNAMES:
        d_w[w] = nc.declare_dram_parameter(w + "_w", [E, E], F32, isOutput=False)
        d_b[w] = nc.declare_dram_parameter(w + "_b", [1, E], F32, isOutput=False)
    d_lng = {}
    d_lnb = {}
    for ln in ["ln_sa", "ln_ff"]:
        d_lng[ln] = nc.declare_dram_parameter(ln + "_g", [1, E], F32, isOutput=False)
        d_lnb[ln] = nc.declare_dram_parameter(ln + "_b", [1, E], F32, isOutput=False)
    d_out = nc.declare_dram_parameter("out", [bl, E], F32, isOutput=True)

    with TileContext(nc) as tc, ExitStack() as ctx:
        const = ctx.enter_context(tc.tile_pool(name="const", bufs=1))
        xpool = ctx.enter_context(tc.tile_pool(name="xpool", bufs=2))
        kpool = ctx.enter_context(tc.tile_pool(name="kpool", bufs=3))
        vpool = ctx.enter_context(tc.tile_pool(name="vpool", bufs=3))
        ppool = ctx.enter_context(tc.tile_pool(name="ppool", bufs=3))
        spool = ctx.enter_context(tc.tile_pool(name="spool", bufs=3))
        acc = ctx.enter_context(tc.tile_pool(name="acc", bufs=2))
        small = ctx.enter_context(tc.tile_pool(name="small", bufs=4))
        psum = ctx.enter_context(tc.tile_pool(name="psum", bufs=2, space="PSUM"))

        # ---- constants ----
        ident = const.tile([128, 128], F32)
        make_identity(nc, ident[:])
        eps_t = const.tile([128, 1], F32)
        nc.vector.memset(eps_t[:], EPS)

        # weights transposed to [e_in, e_out]; biases broadcast to [128, E]
        wt = {}
        bfull = {}
        for w in WNAMES:
            wsb = xpool.tile([E, E], F32, tag="wstage")
            nc.sync.dma_start(out=wsb[:], in_=d_w[w][:])
            pst = psum.tile([E, E], F32, tag="pst")
            nc.tensor.transpose(pst[:], wsb[:], ident[:])
            wt[w] = const.tile([E, E], F32, tag="wt_" + w, name="wt_" + w)
            nc.any.tensor_copy(wt[w][:], pst[:])
            bfull[w] = const.tile([128, E], F32, tag="bf_" + w, name="bf_" + w)
            nc.gpsimd.dma_start(out=bfull[w][:],
                                in_=d_b[w].ap().partition_broadcast(128))
        lngf = {}
        lnbf = {}
        for ln in ["ln_sa", "ln_ff"]:
            lngf[ln] = const.tile([128, E], F32, tag="lng_" + ln, name="lng_" + ln)
            nc.gpsimd.dma_start(out=lngf[ln][:],
                                in_=d_lng[ln].ap().partition_broadcast(128))
            lnbf[ln] = const.tile([128, E], F32, tag="lnb_" + ln, name="lnb_" + ln)
            nc.gpsimd.dma_start(out=lnbf[ln][:],
                                in_=d_lnb[ln].ap().partition_broadcast(128))

        # mask -> negmask f32 (-1e9 where masked); padded to the tile
        # multiple with -1e9 so partial tiles can run the full-width path
        n1_pad = ((n1 + TH - 1) // TH) * TH
        m8 = const.tile([128, n1], U8)
        nc.sync.dma_start(out=m8[:], in_=d_mask[:])
        negmask = const.tile([128, n1_pad], F32)
        nc.vector.tensor_scalar(negmask[:, :n1], m8[:], -1.0e9, None,
                                mybir.AluOpType.mult)
        if n1_pad > n1:
            nc.vector.memset(negmask[:, n1:], -1.0e9)

        # h_t
        ht = const.tile([128, E], F32)
        nc.sync.dma_start(out=ht[:], in_=d_ht[:])

        # ---- helpers ----
        def linear(x, w, out, extra_add=None):
            """out = x @ W^T + b (+ extra_add). x, out: [128, E] sbuf f32."""
            pst = psum.tile([E, E], F32, tag="pst")
            nc.tensor.transpose(pst[:], x[:], ident[:])
            xt = xpool.tile([E, E], F32, tag="xt")
            nc.any.tensor_copy(xt[:], pst[:])
            yps = psum.tile([128, E], F32, tag="yps")
            nc.tensor.matmul(yps[:], xt[:], wt[w][:], start=True, stop=True)
            if extra_add is None:
                nc.vector.tensor_add(out[:], yps[:], bfull[w][:])
            else:
                tmp = xpool.tile([128, E], F32, tag="lin_tmp")
                nc.vector.tensor_add(tmp[:], yps[:], bfull[w][:])
                nc.vector.tensor_add(out[:], tmp[:], extra_add[:])

        def layernorm(x, ln, out):
            stats = small.tile([128, 6], F32, tag="bn_stats")
            nc.vector.bn_stats(stats[:], x[:])
            mv = small.tile([128, 2], F32, tag="bn_mv")
            nc.vector.bn_aggr(mv[:], stats[:])
            std = small.tile([128, 1], F32, tag="std")
            nc.scalar.activation(std[:], mv[:, 1:2],
                                 mybir.ActivationFunctionType.Sqrt,
                                 bias=eps_t[:], scale=1.0)
            rstd = small.tile([128, 1], F32, tag="rstd")
            nc.vector.reciprocal(rstd[:], std[:])
            xn = xpool.tile([128, E], F32, tag="ln_xn")
            nc.vector.tensor_scalar(xn[:], x[:], mv[:, 0:1], rstd[:],
                                    mybir.AluOpType.subtract,
                                    mybir.AluOpType.mult)
            xg = xpool.tile([128, E], F32, tag="ln_xg")
            nc.vector.tensor_mul(xg[:], xn[:], lngf[ln][:])
            nc.vector.tensor_add(out[:], xg[:], lnbf[ln][:])

        def attention(q, kd, vd, s_tot, n_prev, kv_extra, masked, a_out):
            """Batched MHA decode, raw-reshape head semantics.
            q: [128, E] sbuf. kd/vd: dram [bl, n_prev, E] flat-viewed.
            s_tot: total positions per head (n_prev, or n_prev+1 w/ extra).
            kv_extra: None or (k_new, v_new) [128, E] appended at flat end.
            a_out: [128, E] sbuf."""
            nflat_prev = n_prev * E
            ntiles = (s_tot + TH - 1) // TH
            kflat = kd[:].rearrange("b t e -> b (t e)")
            vflat = vd[:].rearrange("b t e -> b (t e)")
            dparts = acc.tile([128, H, ntiles], F32, tag="dparts")
            oparts = acc.tile([128, H, ntiles, HD], F32, tag="oparts")
            for h in range(H):
                qh = q[:, h * HD:(h + 1) * HD]
                qh16 = small.tile([128, HD], F16, tag="qh16", name="qh16")
                nc.vector.tensor_copy(qh16[:], qh)
                for i in range(ntiles):
                    t0 = i * TH
                    ti = min(TH, s_tot - t0)
                    f0 = h * s_tot * HD + t0 * HD
                    avail = max(0, min(ti * HD, nflat_prev - f0))
                    nd = avail // HD  # positions from dram
                    kt = kpool.tile([128, TH, HD], F32, tag="kt")
                    vt = vpool.tile([128, TH, HD], F32, tag="vt")
                    if nd > 0:
                        nc.sync.dma_start(
                            out=kt[:, :nd, :],
                            in_=kflat[:, f0:f0 + nd * HD].rearrange(
                                "b (t d) -> b t d", d=HD))
                        nc.sync.dma_start(
                            out=vt[:, :nd, :],
                            in_=vflat[:, f0:f0 + nd * HD].rearrange(
                                "b (t d) -> b t d", d=HD))
                    if nd < ti:
                        e0 = f0 + nd * HD - nflat_prev
                        ncp = (ti - nd) * HD
                        nc.vector.tensor_copy(
                            kt[:, nd:ti, :],
                            kv_extra[0][:, e0:e0 + ncp].rearrange(
                                "b (t d) -> b t d", d=HD))
                        nc.vector.tensor_copy(
                            vt[:, nd:ti, :],
                            kv_extra[1][:, e0:e0 + ncp].rearrange(
                                "b (t d) -> b t d", d=HD))
                    if mode == "dmaonly":
                        # consume tiles cheaply so DMAs aren't dead-code
                        nc.vector.tensor_copy(dparts[:, h, i:i + 1],
                                              kt[:, 0, 0:1])
                        nc.vector.tensor_copy(oparts[:, h, i, :],
                                              vt[:, 0, :])
                        continue
                    if ti == TH or masked:
                        if ti < TH:
                            # zero the pad so padded scores exp to 0 via the
                            # -1e9 negmask pad (and pv pad is 0 * 0)
                            nc.vector.memset(kt[:, ti:, :], 0.0)
                            nc.vector.memset(vt[:, ti:, :], 0.0)
                        # f16 fast path: 16-bit dtypes unlock the DVE 2x mode.
                        # casts ride ACT (it has its own SBUF port).
                        kth = kpool.tile([128, TH, HD], F16, tag="kth", bufs=2)
                        nc.scalar.activation(
                            kth[:], kt[:],
                            mybir.ActivationFunctionType.Copy)
                        vth = vpool.tile([128, TH, HD], F16, tag="vth", bufs=2)
                        if i % 2 == 0:
                            nc.scalar.activation(
                                vth[:], vt[:],
                                mybir.ActivationFunctionType.Copy)
                        else:
                            nc.vector.tensor_copy(vth[:], vt[:])
                        prod = ppool.tile([128, TH, HD], F16, tag="prod")
                        qb = qh16[:].unsqueeze(1).broadcast_to([128, TH, HD])
                        nc.vector.tensor_mul(prod[:], kth[:], qb)
                        t8 = spool.tile([128, TH, 8], F16, tag="t8")
                        nc.vector.tensor_add(t8[:], prod[:, :, 0:8],
                                             prod[:, :, 8:16])
                        t4 = spool.tile([128, TH, 4], F16, tag="t4")
                        nc.vector.tensor_add(t4[:], t8[:, :, 0:4],
                                             t8[:, :, 4:8])
                        t2 = spool.tile([128, TH, 2], F16, tag="t2")
                        nc.vector.tensor_add(t2[:], t4[:, :, 0:2],
                                             t4[:, :, 2:4])
                        s_raw = spool.tile([128, TH], F32, tag="s_raw")
                        nc.vector.tensor_add(s_raw[:], t2[:, :, 0],
                                             t2[:, :, 1])
                        p16 = spool.tile([128, TH], F16, tag="p16")
                        if masked:
                            s_msk = spool.tile([128, TH], F32, tag="s_msk")
                            nc.vector.scalar_tensor_tensor(
                                s_msk[:], s_raw[:], 0.25,
                                negmask[:, t0:t0 + TH],
                                mybir.AluOpType.mult, mybir.AluOpType.add)
                            nc.scalar.activation(
                                p16[:], s_msk[:],
                                mybir.ActivationFunctionType.Exp,
                                accum_out=dparts[:, h, i:i + 1])
                        else:
                            nc.scalar.activation(
                                p16[:], s_raw[:],
                                mybir.ActivationFunctionType.Exp,
                                scale=0.25,
                                accum_out=dparts[:, h, i:i + 1])
                        pv = ppool.tile([128, TH, HD], F16, tag="pv")
                        pb = p16[:].unsqueeze(2).broadcast_to([128, TH, HD])
                        nc.vector.tensor_mul(pv[:], vth[:], pb)
                        pv2 = ppool.tile([128, TH // 2, HD], F16, tag="pv2", bufs=2)
                        nc.vector.tensor_add(pv2[:], pv[:, 0:TH // 2, :],
                                             pv[:, TH // 2:TH, :])
                        pv3 = ppool.tile([128, TH // 4, HD], F16, tag="pv3", bufs=2)
                        nc.vector.tensor_add(pv3[:], pv2[:, 0:TH // 4, :],
                                             pv2[:, TH // 4:TH // 2, :])
                        pv4 = ppool.tile([128, TH // 8, HD], F16, tag="pv4", bufs=2)
                        nc.vector.tensor_add(pv4[:], pv3[:, 0:TH // 8, :],
                                             pv3[:, TH // 8:TH // 4, :])
                        nc.vector.tensor_reduce(
                            oparts[:, h, i, :], pv4[:].transpose([0, 2, 1]),
                            mybir.AxisListType.X, mybir.AluOpType.add)
                        continue
                    # f32 remainder path (partial tiles)
                    prod = ppool.tile([128, TH, HD], F32, tag="prodf", bufs=1)
                    qb = qh.unsqueeze(1).broadcast_to([128, ti, HD])
                    nc.vector.tensor_mul(prod[:, :ti, :], kt[:, :ti, :], qb)
                    s_raw = spool.tile([128, TH], F32, tag="s_rawf", bufs=1)
                    nc.vector.tensor_reduce(s_raw[:, :ti], prod[:, :ti, :],
                                            mybir.AxisListType.X,
                                            mybir.AluOpType.add)
                    p = spool.tile([128, TH], F32, tag="p")
                    if masked:
                        s_msk = spool.tile([128, TH], F32, tag="s_mskf", bufs=1)
                        nc.vector.scalar_tensor_tensor(
                            s_msk[:, :ti], s_raw[:, :ti], 0.25,
                            negmask[:, t0:t0 + ti],
                            mybir.AluOpType.mult, mybir.AluOpType.add)
                        nc.scalar.activation(p[:, :ti], s_msk[:, :ti],
                                             mybir.ActivationFunctionType.Exp,
                                             accum_out=dparts[:, h, i:i + 1])
                    else:
                        nc.scalar.activation(p[:, :ti], s_raw[:, :ti],
                                             mybir.ActivationFunctionType.Exp,
                                             scale=0.25,
                                             accum_out=dparts[:, h, i:i + 1])
                    pv = ppool.tile([128, TH, HD], F32, tag="pvf", bufs=1)
                    pb = p[:, :ti].unsqueeze(2).broadcast_to([128, ti, HD])
                    nc.vector.tensor_mul(pv[:, :ti, :], vt[:, :ti, :], pb)
                    nc.vector.tensor_reduce(oparts[:, h, i, :],
                                            pv[:, :ti, :].transpose([0, 2, 1]),
                                            mybir.AxisListType.X,
                                            mybir.AluOpType.add)
            d = small.tile([128, H], F32, tag="attn_d")
            nc.vector.tensor_reduce(d[:], dparts[:], mybir.AxisListType.X,
                                    mybir.AluOpType.add)
            r = small.tile([128, H], F32, tag="attn_r")
            nc.vector.reciprocal(r[:], d[:])
            o = xpool.tile([128, E], F32, tag="attn_o")
            nc.vector.tensor_reduce(o[:].rearrange("p (h d) -> p h d", h=H),
                                    oparts[:].transpose([0, 1, 3, 2]),
                                    mybir.AxisListType.X, mybir.AluOpType.add)
            rb = r[:].unsqueeze(2).broadcast_to([128, H, HD])
            nc.vector.tensor_mul(
                a_out[:].rearrange("p (h d) -> p h d", h=H),
                o[:].rearrange("p (h d) -> p h d", h=H), rb)

        # ---- model ----
        for _rep in range(repeat):
            k_sa = xpool.tile([128, E], F32, tag="k_sa", name="k_sa")
            linear(ht, "Wk", k_sa)
            v_sa = xpool.tile([128, E], F32, tag="v_sa", name="v_sa")
            linear(ht, "Wv", v_sa)

            a_sa = xpool.tile([128, E], F32, tag="a_sa", name="a_sa")
            attention(ht, d_ksa, d_vsa, t_prev + 1, t_prev, (k_sa, v_sa),
                      False, a_sa)

            h1 = xpool.tile([128, E], F32, tag="h1", name="h1")
            linear(a_sa, "W0sa", h1, extra_add=ht)
            h1ln = xpool.tile([128, E], F32, tag="h1ln", name="h1ln")
            layernorm(h1, "ln_sa", h1ln)

            q = xpool.tile([128, E], F32, tag="q", name="q")
            linear(h1ln, "Wqatt", q)
            a_att = xpool.tile([128, E], F32, tag="a_att", name="a_att")
            attention(q, d_katt, d_vatt, n1, n1, None, True, a_att)

            h2 = xpool.tile([128, E], F32, tag="h2", name="h2")
            linear(a_att, "W0att", h2, extra_add=h1ln)
            h2ln = xpool.tile([128, E], F32, tag="h2ln", name="h2ln")
            layernorm(h2, "ln_sa", h2ln)

            ff_pre = xpool.tile([128, E], F32, tag="ff_pre", name="ff_pre")
            linear(h2ln, "W1", ff_pre)
            ff = xpool.tile([128, E], F32, tag="ff", name="ff")
            nc.scalar.activation(ff[:], ff_pre[:],
                                 mybir.ActivationFunctionType.Relu)
            h3 = xpool.tile([128, E], F32, tag="h3", name="h3")
            linear(ff, "W2", h3, extra_add=h2ln)
            h3ln = xpool.tile([128, E], F32, tag="h3ln", name="h3ln")
            layernorm(h3, "ln_ff", h3ln)

            nc.sync.dma_start(out=d_out[:], in_=h3ln[:])

    nc.compile()
    return nc


_NC_CACHE = {}


def _get_nc():
    key = (BL, N1, T_PREV)
    if key not in _NC_CACHE:
        _NC_CACHE[key] = build_kernel()
    return _NC_CACHE[key]


def make_in_maps(inputs, bl=BL, ncores=NCORES):
    """Shard batch dim across cores; replicate weights."""
    in_maps = []
    for c in range(ncores):
        sl = slice(c * bl, (c + 1) * bl)
        m = {}
        m["h_t"] = np.ascontiguousarray(
            inputs["h_t"][sl].reshape(bl, E).astype(np.float32))
        for k in ["K_att", "V_att", "K_sa_prev", "V_sa_prev"]:
            m[k] = np.ascontiguousarray(inputs[k][sl].astype(np.float32))
        m["mask"] = np.ascontiguousarray(inputs["mask"][sl].astype(np.uint8))
        for w in WNAMES:
            m[w + "_w"] = np.ascontiguousarray(inputs[w + "_w"].astype(np.float32))
            m[w + "_b"] = np.ascontiguousarray(
                inputs[w + "_b"].reshape(1, E).astype(np.float32))
        for ln in ["ln_sa", "ln_ff"]:
            m[ln + "_g"] = np.ascontiguousarray(
                inputs[ln + "_g"].reshape(1, E).astype(np.float32))
            m[ln + "_b"] = np.ascontiguousarray(
                inputs[ln + "_b"].reshape(1, E).astype(np.float32))
        in_maps.append(m)
    return in_maps


def kernel(**inputs):
    nc = _get_nc()
    in_maps = make_in_maps(inputs)
    res = run_bass_kernel_spmd(nc, in_maps, core_ids=list(range(NCORES)))
    outs = [res.results[i]["out"].reshape(BL, 1, E) for i in range(NCORES)]
    return np.concatenate(outs, axis=0)
